# revision 1
# baseline (speedup 1.0000x reference)
"""Bipartite graph convolution (GCMC-style) Trainium2 kernel, 8-core SPMD.

Math (reference): per-rating masks M_r = (adj == r), r=1..5,
  out_u = relu(d_u * sum_r (M_r @ v_feat) @ W_u[r]),  d_u = 1/deg_u
  out_v = relu(d_v * sum_r (M_r.T @ u_feat) @ W_v[r]), d_v = 1/deg_v

Device formulation (per core, u-rows sharded 1024/core):
  Fold weights on host: P_r = v_feat @ W_u[r], Q_r = u_feat_shard @ W_v[r].
  Basis trick: since adj = sum_r r*M_r,
    sum_r M_r X_r = adj @ X_1 + sum_{r=2..5} M_r @ (X_r - r*X_1)
  so only 4 on-chip is_equal mask passes per orientation (adj tile itself is
  the 5th stationary operand). A 65th moving column carries per-basis
  constants (+1 for adj, -(r-1) for M_r) so PSUM col 64 accumulates the
  degree (edge count) for free.
  PE: stationary = [128u x 128v] fp16 mask/adj tile, moving = [128, 65]
  fp16 features+deg -> PSUM f32 [128, 65].
  Phase A (out_u): lhsT = adjT tiles (streamed), 8 persistent PSUM banks
  accumulate the whole u-shard; finish deg/relu on-chip.
  Phase B (out_v): lhsT = adj tiles (resident), 8 PSUM banks per v-group of
  8, partial [8192, 65] DMA'd out; host all-reduces over cores + finishes.
"""

import numpy as np
import sys

sys.path.insert(0, "/opt/trn_rl_repo")

N_U, N_V = 8192, 8192
F = 64
R = 5
N_CORES = 8
U_SH = N_U // N_CORES          # 1024 rows per core
UC = U_SH // 128               # 8 u-chunks per core
VC = N_V // 128                # 64 v-chunks
VG = 8                         # v-groups of 8 chunks (phase B)
J = F + 1                      # 64 features + degree column

_CACHE = {}


def _build():
    import concourse.bass as bass
    import concourse.bacc as bacc
    import concourse.mybir as mybir
    import concourse.tile as tile

    dt = mybir.dt
    eq = mybir.AluOpType.is_equal
    mx = mybir.AluOpType.max
    mult = mybir.AluOpType.mult
    SQ = mybir.ActivationFunctionType.Square
    RELU = mybir.ActivationFunctionType.Relu

    nc = bacc.Bacc("TRN2", target_bir_lowering=False, debug=False,
                   num_devices=N_CORES)

    adj_h = nc.dram_tensor("adj_h", [U_SH, N_V], dt.float16,
                           kind="ExternalInput").ap()
    adjt_h = nc.dram_tensor("adjt_h", [N_V, U_SH], dt.float16,
                            kind="ExternalInput").ap()
    q_mov = nc.dram_tensor("q_mov", [128, R * UC * J], dt.float16,
                           kind="ExternalInput").ap()
    p_mov = nc.dram_tensor("p_mov", [128, R * VC * J], dt.float16,
                           kind="ExternalInput").ap()
    out_u = nc.dram_tensor("out_u_part", [U_SH, F], dt.float32,
                           kind="ExternalOutput").ap()
    out_v = nc.dram_tensor("out_v_part", [N_V, J], dt.float32,
                           kind="ExternalOutput").ap()

    def gen_masks(nc, mtiles, src, W, bias_m3):
        """Basis tiles for ratings 2..5 of src [128, W]. DVE: one-hot
        r=2,3,5 into mt_d (r2|r3|r5); ACT: ramp4 = relu(a-3) (values
        {0,1,2}, exact) into mt_a in a single op. The host moving
        tensors are solved for basis {adj, M2, M3, ramp4, M5}."""
        mt_d, mt_a = mtiles
        nc.vector.tensor_scalar(mt_d[:, 0:W], src, 2.0, None, op0=eq)
        nc.vector.tensor_scalar(mt_d[:, W:2 * W], src, 3.0, None, op0=eq)
        nc.vector.tensor_scalar(mt_d[:, 2 * W:3 * W], src, 5.0, None, op0=eq)
        nc.scalar.activation(mt_a[:], src, RELU, bias=bias_m3[:, 0:1])

    with tile.TileContext(nc) as tc:
        with tc.tile_pool(name="consts", bufs=1) as cons, \
             tc.tile_pool(name="adjres", bufs=1) as adjres, \
             tc.tile_pool(name="fin", bufs=4) as fin:

            # SBUF/partition budget (192K cap): adj resident 128K + adjt
            # stream 4x2K + masks 2x(12+4+4)K + pstream 4x.7K + q 5.1K
            q_t = cons.tile([128, R * UC * J], dt.float16, tag="q")
            bias_m3 = cons.tile([128, 1], dt.float32, tag="bm3")
            nc.gpsimd.memset(bias_m3[:], -3.0)
            # warm the ACT spline table during initial DMA wait
            warm = cons.tile([128, 1], dt.float16, tag="warm")
            nc.scalar.activation(warm[:], bias_m3[:], RELU)
            zt = cons.tile([128, 4 * J], dt.float16, tag="zt")
            nc.gpsimd.memset(zt[:], 0.0)

            adj_q = [[adjres.tile([128, 2048], dt.float16,
                                  tag=f"adj{uc}_{q}", name=f"adjr{uc}_{q}")
                      for q in range(4)] for uc in range(UC)]

            def mask_tiles(pool, W, key):
                mt_d = pool.tile([128, 3 * W], dt.float16, tag="mtd",
                                 name=f"mtd{key}")
                mt_a = pool.tile([128, W], dt.float16, tag="mta",
                                 name=f"mta{key}")
                return mt_d, mt_a

            def lhsT_of(mtiles, base, W, b, i):
                mt_d, mt_a = mtiles
                if b == 0:
                    return base[:, i * 128:(i + 1) * 128]
                if b == 3:
                    return mt_a[:, i * 128:(i + 1) * 128]
                seg = {1: 0, 2: 1, 4: 2}[b]
                return mt_d[:, seg * W + i * 128:seg * W + (i + 1) * 128]

            # ---------------- Phase A: out_u ----------------
            pspA = tc.tile_pool(name="psumA", bufs=1, space="PSUM")
            psp = pspA.__enter__()
            mpoolA = tc.tile_pool(name="maskA", bufs=4)
            mpa = mpoolA.__enter__()
            adjtp = tc.tile_pool(name="adjts", bufs=6)
            adjts = adjtp.__enter__()
            ps_u = [psp.tile([128, J], dt.float32, tag=f"psu{uc}",
                             name=f"psu{uc}") for uc in range(UC)]
            # only the first column-quarter of each adj row-block loads
            # in phase A (needed by phase B's first v-group); later
            # quarters stream during phase B one v-group ahead
            adj_sched = {30 + k * 4: k * 4 for k in range(UC)}
            for vc in range(VC):
                at = adjts.tile([128, U_SH], dt.float16, tag="adjt")
                nc.sync.dma_start(at[:], adjt_h[vc * 128:(vc + 1) * 128, :])
                pt = mpa.tile([128, R * J], dt.float16, tag="pstream",
                              bufs=4)
                nc.sync.dma_start(pt[:], p_mov[:, vc * R * J:(vc + 1) * R * J])
                # spread the 16MB resident-adj load through phase A in
                # 512KB quarters so it never head-of-line-blocks streams
                k = adj_sched.get(vc)
                if k is not None:
                    uc = k // 4
                    nc.sync.dma_start(
                        adj_q[uc][0][:],
                        adj_h[uc * 128:(uc + 1) * 128, 0:2048])
                if vc == 4:
                    nc.sync.dma_start(q_t[:], q_mov[:])
                mtiles = mask_tiles(mpa, U_SH, f"a{vc}")
                gen_masks(nc, mtiles, at[:], U_SH, bias_m3)
                for uc in range(UC):
                    for b in range(R):
                        nc.tensor.matmul(
                            ps_u[uc][:], lhsT_of(mtiles, at, U_SH, b, uc),
                            pt[:, b * J:(b + 1) * J],
                            start=(vc == 0 and b == 0),
                            stop=(vc == VC - 1 and b == R - 1))
            # finish out_u: d_u = 1/max(deg,0.5); relu(d_u * x) on ACT
            for uc in range(UC):
                dtl = fin.tile([128, 1], dt.float32, tag="deg")
                nc.vector.tensor_scalar(dtl[:], ps_u[uc][:, F:F + 1], 0.5,
                                        None, op0=mx)
                rtl = fin.tile([128, 1], dt.float32, tag="rec")
                nc.vector.reciprocal(rtl[:], dtl[:])
                otl = fin.tile([128, F], dt.float32, tag="outu")
                nc.scalar.activation(otl[:], ps_u[uc][:, 0:F], RELU,
                                     scale=rtl[:, 0:1])
                nc.sync.dma_start(out_u[uc * 128:(uc + 1) * 128, :], otl[:])

            adjtp.__exit__(None, None, None)
            mpoolA.__exit__(None, None, None)
            pspA.__exit__(None, None, None)

            # ------- Phase B: out_v partial (2 accumulators per bank) -----
            pspB = tc.tile_pool(name="psumB", bufs=1, space="PSUM")
            psp = pspB.__enter__()
            mpoolB = tc.tile_pool(name="maskB", bufs=3)
            mpb = mpoolB.__enter__()
            W2 = 2048
            for vg2 in range(4):
                ps2 = [psp.tile([128, 4 * J], dt.float32, tag=f"psv{k}",
                                name=f"psv{vg2}_{k}", bufs=2)
                       for k in range(4)]
                # dummy start=True matmul zeroes all 4 slots & owns the
                # bank-wide has_written clear; real matmuls accumulate
                for k in range(4):
                    nc.tensor.matmul(ps2[k][:], q_t[:, 0:128], zt[:],
                                     start=True, stop=False,
                                     skip_group_check=True)
                for uc in range(UC):
                    if vg2 < 3:
                        nc.sync.dma_start(
                            adj_q[uc][vg2 + 1][:],
                            adj_h[uc * 128:(uc + 1) * 128,
                                  (vg2 + 1) * W2:(vg2 + 2) * W2])
                    src = adj_q[uc][vg2][:]
                    mtiles = mask_tiles(mpb, W2, f"b{vg2}_{uc}")
                    gen_masks(nc, mtiles, src, W2, bias_m3)
                    for i in range(16):
                        k, sl = i % 4, i // 4
                        for b in range(R):
                            nc.tensor.matmul(
                                ps2[k][:, sl * J:(sl + 1) * J],
                                lhsT_of(mtiles, src, W2, b, i),
                                q_t[:, (b * UC + uc) * J:
                                    (b * UC + uc + 1) * J],
                                start=False,
                                stop=(uc == UC - 1 and b == R - 1
                                      and sl == 3),
                                skip_group_check=True)
                for k in range(4):
                    ev = fin.tile([128, 4 * J], dt.float32, tag="evac",
                                  name=f"ev{vg2}_{k}")
                    nc.scalar.copy(ev[:], ps2[k][:])
                    for sl in range(4):
                        vc = vg2 * 16 + sl * 4 + k
                        nc.sync.dma_start(
                            out_v[vc * 128:(vc + 1) * 128, :],
                            ev[:, sl * J:(sl + 1) * J])
            mpoolB.__exit__(None, None, None)
            pspB.__exit__(None, None, None)

    nc.compile()
    return nc


def _host_prep(adj, u_feature, v_feature, weight_u, weight_v):
    adj = np.asarray(adj)
    u_feature = np.asarray(u_feature, dtype=np.float32)
    v_feature = np.asarray(v_feature, dtype=np.float32)
    weight_u = np.asarray(weight_u, dtype=np.float32)
    weight_v = np.asarray(weight_v, dtype=np.float32)

    adj16 = adj.astype(np.float16)

    # P_r = v_feat @ W_u[r]  (phase A moving), Q_r = u_shard @ W_v[r] (phase B)
    P = np.einsum("vf,rfo->rvo", v_feature, weight_u)      # [R, N_V, F]
    # basis transform: X^_1 = X_1 ; X^_r = X_r - r*X_1 (r=2..5)
    Pb = np.empty((R, N_V, J), np.float32)
    Pb[0, :, :F] = P[0]
    Pb[0, :, F] = 1.0
    for r in range(2, R + 1):
        Pb[r - 1, :, :F] = P[r - 1] - r * P[0]
        Pb[r - 1, :, F] = -(r - 1)
    # basis element 3 is ramp4=relu(a-3) (not M4); element 4 pairs M5 with
    # X5 + 3*X1 - 2*X4 and degree-coefficient +2 so the span is unchanged
    Pb[4, :, :F] = P[4] + 3.0 * P[0] - 2.0 * P[3]
    Pb[4, :, F] = 2.0
    # p_mov[p, (vc*R+b)*J + j] = Pb[b, vc*128+p, j]  (vc-major for streaming)
    p_mov = np.ascontiguousarray(
        Pb.reshape(R, VC, 128, J).transpose(2, 1, 0, 3).reshape(128, R * VC * J)
    ).astype(np.float16)

    in_maps = []
    for c in range(N_CORES):
        sl = slice(c * U_SH, (c + 1) * U_SH)
        Q = np.einsum("uf,rfo->ruo", u_feature[sl], weight_v)  # [R, U_SH, F]
        Qb = np.empty((R, U_SH, J), np.float32)
        Qb[0, :, :F] = Q[0]
        Qb[0, :, F] = 1.0
        for r in range(2, R + 1):
            Qb[r - 1, :, :F] = Q[r - 1] - r * Q[0]
            Qb[r - 1, :, F] = -(r - 1)
        Qb[4, :, :F] = Q[4] + 3.0 * Q[0] - 2.0 * Q[3]
        Qb[4, :, F] = 2.0
        q_mov = np.ascontiguousarray(
            Qb.reshape(R, UC, 128, J).transpose(2, 0, 1, 3)
            .reshape(128, R * UC * J)).astype(np.float16)
        a = adj16[sl]
        in_maps.append({
            "adj_h": np.ascontiguousarray(a),
            "adjt_h": np.ascontiguousarray(a.T),
            "q_mov": q_mov,
            "p_mov": p_mov,
        })
    return in_maps


def kernel(adj, u_feature, v_feature, weight_u, weight_v, _trace=False):
    from concourse import bass_utils

    if "nc" not in _CACHE:
        _CACHE["nc"] = _build()
    nc = _CACHE["nc"]

    in_maps = _host_prep(adj, u_feature, v_feature, weight_u, weight_v)
    res = bass_utils.run_bass_kernel_spmd(
        nc, in_maps, core_ids=list(range(N_CORES)), trace=_trace)
    _CACHE["last_result"] = res

    out_u = np.concatenate([res.results[c]["out_u_part"]
                            for c in range(N_CORES)], axis=0)
    acc = np.zeros((N_V, J), np.float64)
    for c in range(N_CORES):
        acc += res.results[c]["out_v_part"]
    acc = acc.astype(np.float32)
    deg_v = acc[:, F]
    d_v = np.where(deg_v > 0, 1.0 / np.maximum(deg_v, 0.5), 0.0)
    out_v = np.maximum(acc[:, :F] * d_v[:, None], 0.0).astype(np.float32)
    return out_u, out_v



# revision 6
# speedup vs baseline: 1.0318x; 1.0318x over previous
"""Bipartite graph convolution (GCMC-style) Trainium2 kernel, 8-core SPMD.

Math (reference): per-rating masks M_r = (adj == r), r=1..5,
  out_u = relu(d_u * sum_r (M_r @ v_feat) @ W_u[r]),  d_u = 1/deg_u
  out_v = relu(d_v * sum_r (M_r.T @ u_feat) @ W_v[r]), d_v = 1/deg_v

Device formulation (per core, u-rows sharded 1024/core), v2:
  Fold weights on host: P_b = v_feat @ W~_u[b]  [8192, 64] per basis b,
  Q_b = u_shard @ W~_v[b] [1024, 64], where basis {adj, M2, M3, ramp4, M5}
  replaces the one-hot masks (adj = sum_r r*M_r) so only 4 on-chip mask
  passes per orientation (3 DVE is_equal + 1 ACT relu; adj streams as-is).

  PE uses feature-stationary matmuls with 4-way column tiling: per span,
  4 concurrent MMs with 32-col stationaries (two bases x two 32-feature
  halves) stream 512-wide mask columns -> 4 moving cols/cycle, full
  128x128 array utilization. Output is feature-major [f, cols] in PSUM;
  all bases accumulate into one bank (basis pairs split across partition
  rows 0:64 / 64:128, summed on host). A dummy start=True matmul with a
  zero moving tile owns each bank's has_written clear (col-tiled groups
  run concurrently, so no real MM may clear the bank).

  Phase 1 (out_v partial): adj row-tiles streamed [128u, 4096v] per
  (vbg, uc-pair); per v-block of 512, one PSUM bank accumulates over all
  uc and bases; partial [128, 8192] fp16 -> host (all-reduce over cores
  + halves-add + deg + relu).
  Phase 2 (out_u): adjT tiles streamed [128v, 1024u]; 2 PSUM banks
  (u-halves) accumulate over all 64 v-tiles; partial [128, 2048] fp16
  -> host (halves-add + deg + relu).
"""

import numpy as np
import sys

sys.path.insert(0, "/opt/trn_rl_repo")

N_U, N_V = 8192, 8192
F = 64
R = 5
N_CORES = 8
U_SH = N_U // N_CORES          # 1024 rows per core
UC = U_SH // 128               # 8 u-chunks per core
VC = N_V // 128                # 64 v-chunks
VBW = 512                      # v-block width (phase 1 psum bank)
NVB = N_V // VBW               # 16 v-blocks
VBG = 2                        # v-block groups (8 banks each)
HB = NVB // VBG                # 8 v-blocks per group
W1 = N_V // VBG                # 4096 adj cols per phase-1 slice

_CACHE = {}


def _build():
    import concourse.bass as bass
    import concourse.bacc as bacc
    import concourse.mybir as mybir
    import concourse.tile as tile

    dt = mybir.dt
    eq = mybir.AluOpType.is_equal
    RELU = mybir.ActivationFunctionType.Relu

    nc = bacc.Bacc("TRN2", target_bir_lowering=False, debug=False,
                   num_devices=N_CORES)

    adj_h = nc.dram_tensor("adj_h", [U_SH, N_V], dt.float16,
                           kind="ExternalInput").ap()
    adjt_h = nc.dram_tensor("adjt_h", [N_V, U_SH], dt.float16,
                            kind="ExternalInput").ap()
    # stationaries: per-chunk feature projections, 5 bases x 64 features
    q_stat_h = nc.dram_tensor("q_stat_h", [128, UC * R * F], dt.float16,
                              kind="ExternalInput").ap()
    p_stat_h = nc.dram_tensor("p_stat_h", [128, VC * R * F], dt.float16,
                              kind="ExternalInput").ap()
    out_u = nc.dram_tensor("out_u_part", [128, 2 * VBW], dt.float16,
                           kind="ExternalOutput").ap()
    out_v = nc.dram_tensor("out_v_part", [128, N_V], dt.float16,
                           kind="ExternalOutput").ap()

    with tile.TileContext(nc) as tc:
        with tc.tile_pool(name="consts", bufs=1) as cons, \
             tc.tile_pool(name="fin", bufs=4) as fin:

            bias_m3 = cons.tile([128, 1], dt.float32, tag="bm3")
            nc.gpsimd.memset(bias_m3[:], -3.0)
            # warm the ACT spline table during initial DMA wait
            warm = cons.tile([128, 1], dt.float16, tag="warm")
            nc.scalar.activation(warm[:], bias_m3[:], RELU)
            zt = cons.tile([128, VBW], dt.float16, tag="zt")
            nc.gpsimd.memset(zt[:], 0.0)

            q_t = cons.tile([128, UC * R * F], dt.float16, tag="q")
            nc.sync.dma_start(q_t[:], q_stat_h[:])

            def gen_masks(pool, src, W, key):
                """4 basis tiles from src [128, W]: M2/M3/M5 on DVE
                (is_equal), ramp4 = relu(a-3) on ACT. Returns accessor
                b, lo, hi -> AP: basis {0:adj, 1:M2, 2:M3, 3:r4, 4:M5}."""
                mt = pool.tile([128, 3 * W], dt.float16, tag=f"mtd{key[0]}",
                               name=f"mtd{key}")
                ma = pool.tile([128, W], dt.float16, tag=f"mta{key[0]}",
                               name=f"mta{key}")
                nc.vector.tensor_scalar(mt[:, 0:W], src, 2.0, None, op0=eq)
                nc.vector.tensor_scalar(mt[:, W:2 * W], src, 3.0, None,
                                        op0=eq)
                nc.vector.tensor_scalar(mt[:, 2 * W:3 * W], src, 5.0, None,
                                        op0=eq)
                nc.scalar.activation(ma[:], src, RELU, bias=bias_m3[:, 0:1])

                def basis(b, lo, hi):
                    if b == 0:
                        return src[:, lo:hi]
                    if b == 3:
                        return ma[:, lo:hi]
                    seg = {1: 0, 2: 1, 4: 2}[b]
                    return mt[:, seg * W + lo:seg * W + hi]
                return basis

            def clear_bank(ps_tile):
                """Dummy start=True matmul: zeros the bank, sets
                has_written everywhere; real MMs accumulate (start=False).
                """
                nc.tensor.matmul(ps_tile[:], zt[:, 0:128], zt[:],
                                 start=True, stop=False,
                                 skip_group_check=True)

            def mm_grp(ps_tile, g, lhsT, rhs, stop=False):
                nc.tensor.matmul(ps_tile[32 * g:32 * (g + 1), :],
                                 lhsT, rhs,
                                 start=False, stop=stop,
                                 tile_position=(0, 32 * g),
                                 skip_group_check=True)

            def span(ps_tile, rhs_e, lhs_e, rhs_o, lhs_o, stop=False):
                """4-MM col-tiled span: groups 0/1 = even-basis feature
                halves vs rhs_e, groups 2/3 = odd basis vs rhs_o.
                lhs_*: (stationary tile, col offset of 64-wide block)."""
                (te, ce), (to, co) = lhs_e, lhs_o
                for g in range(4):
                    t, c0 = (te, ce) if g < 2 else (to, co)
                    rhs = rhs_e if g < 2 else rhs_o
                    fh = g % 2
                    mm_grp(ps_tile, g, t[:, c0 + 32 * fh:c0 + 32 * (fh + 1)],
                           rhs, stop=(stop and g == 3))

            # ---------------- Phase 1: out_v partial ----------------
            pspB = tc.tile_pool(name="psumB", bufs=1, space="PSUM")
            psp = pspB.__enter__()
            mpoolB = tc.tile_pool(name="maskB", bufs=2)
            mpb = mpoolB.__enter__()
            ps_v = [psp.tile([128, VBW], dt.float32, tag=f"psv{k}",
                             name=f"psv{k}") for k in range(HB)]

            for vbg in range(VBG):
                for vb in range(HB):
                    clear_bank(ps_v[vb])
                for ucp in range(UC // 2):
                    masks = []
                    for i in range(2):
                        uc = 2 * ucp + i
                        at = mpb.tile([128, W1], dt.float16,
                                      tag=f"adj{i}", name=f"a{vbg}_{uc}")
                        nc.sync.dma_start(
                            at[:], adj_h[uc * 128:(uc + 1) * 128,
                                         vbg * W1:(vbg + 1) * W1])
                        basis = gen_masks(mpb, at[:], W1, (i, vbg, uc))
                        masks.append((uc, basis))
                    last_ucp = ucp == UC // 2 - 1
                    for vb in range(HB):
                        lo, hi = vb * VBW, (vb + 1) * VBW
                        for pr in range(2):  # pairs (adj,M2), (M3,r4)
                            for uc, basis in masks:
                                qc = uc * R * F
                                span(ps_v[vb],
                                     basis(2 * pr, lo, hi),
                                     (q_t, qc + (2 * pr) * F),
                                     basis(2 * pr + 1, lo, hi),
                                     (q_t, qc + (2 * pr + 1) * F))
                        # pair3: M5 of uc-even on grp 0/1, uc-odd on 2/3
                        (uca, ba), (ucb, bb) = masks
                        span(ps_v[vb],
                             ba(4, lo, hi), (q_t, uca * R * F + 4 * F),
                             bb(4, lo, hi), (q_t, ucb * R * F + 4 * F),
                             stop=last_ucp)
                # evacuate the 8 banks of this vbg (fp16 partial out)
                for vb in range(HB):
                    ev = fin.tile([128, VBW], dt.float16, tag="evacv",
                                  name=f"evv{vbg}_{vb}")
                    nc.scalar.copy(ev[:], ps_v[vb][:])
                    nc.sync.dma_start(
                        out_v[:, vbg * W1 + vb * VBW:
                              vbg * W1 + (vb + 1) * VBW], ev[:])
            mpoolB.__exit__(None, None, None)
            pspB.__exit__(None, None, None)

            # ---------------- Phase 2: out_u partial ----------------
            pspA = tc.tile_pool(name="psumA", bufs=1, space="PSUM")
            psp = pspA.__enter__()
            mpoolA = tc.tile_pool(name="maskA", bufs=3)
            mpa = mpoolA.__enter__()
            pstp = tc.tile_pool(name="pstream", bufs=3)
            pst = pstp.__enter__()
            ps_u = [psp.tile([128, VBW], dt.float32, tag=f"psu{h}",
                             name=f"psu{h}") for h in range(2)]
            for h in range(2):
                clear_bank(ps_u[h])
            for vc in range(VC):
                at = mpa.tile([128, U_SH], dt.float16, tag="adjt",
                              name=f"at{vc}")
                nc.sync.dma_start(at[:], adjt_h[vc * 128:(vc + 1) * 128, :])
                pt = pst.tile([128, R * F], dt.float16, tag="pstat",
                              name=f"pt{vc}")
                nc.sync.dma_start(pt[:],
                                  p_stat_h[:, vc * R * F:(vc + 1) * R * F])
                basis = gen_masks(mpa, at[:], U_SH, ("p", vc))
                last = vc == VC - 1
                for h in range(2):
                    lo, hi = h * VBW, (h + 1) * VBW
                    for pr in range(2):
                        span(ps_u[h],
                             basis(2 * pr, lo, hi), (pt, (2 * pr) * F),
                             basis(2 * pr + 1, lo, hi),
                             (pt, (2 * pr + 1) * F))
                # pair3: M5 u-half0 on grp 0/1 (bank 0 rows 0:64),
                #        M5 u-half1 on grp 2/3 (bank 1 rows 64:128)
                for g in range(4):
                    h = g // 2
                    fh = g % 2
                    mm_grp(ps_u[h], g,
                           pt[:, 4 * F + 32 * fh:4 * F + 32 * (fh + 1)],
                           basis(4, h * VBW, (h + 1) * VBW),
                           stop=(last and fh == 1))
            for h in range(2):
                ev = fin.tile([128, VBW], dt.float16, tag="evacu",
                              name=f"evu{h}")
                nc.scalar.copy(ev[:], ps_u[h][:])
                nc.sync.dma_start(out_u[:, h * VBW:(h + 1) * VBW], ev[:])
            pstp.__exit__(None, None, None)
            mpoolA.__exit__(None, None, None)
            pspA.__exit__(None, None, None)

    nc.compile()
    return nc


def _host_prep(adj, u_feature, v_feature, weight_u, weight_v):
    adj = np.asarray(adj)
    u_feature = np.asarray(u_feature, dtype=np.float32)
    v_feature = np.asarray(v_feature, dtype=np.float32)
    weight_u = np.asarray(weight_u, dtype=np.float32)
    weight_v = np.asarray(weight_v, dtype=np.float32)

    adj16 = adj.astype(np.float16)

    def basis_fold(X):
        """X [R, n, F] per-rating projections -> basis projections for
        {adj, M2, M3, ramp4, M5}: sum_r M_r X_r = a*X1 + M2*(X2-2X1)
        + M3*(X3-3X1) + r4*(X4-4X1) + M5*(X5+3X1-2X4)   (r4(5)=2)."""
        B = np.empty_like(X)
        B[0] = X[0]
        B[1] = X[1] - 2 * X[0]
        B[2] = X[2] - 3 * X[0]
        B[3] = X[3] - 4 * X[0]
        B[4] = X[4] + 3 * X[0] - 2 * X[3]
        return B

    # P_b = v_feat @ W_u basis-folded  [R, N_V, F]
    P = basis_fold(np.einsum("vf,rfo->rvo", v_feature, weight_u))
    # p_stat[p, (vc*R+b)*F + j] = P[b, vc*128+p, j]
    p_stat = np.ascontiguousarray(
        P.reshape(R, VC, 128, F).transpose(2, 1, 0, 3).reshape(128, -1)
    ).astype(np.float16)

    in_maps = []
    for c in range(N_CORES):
        sl = slice(c * U_SH, (c + 1) * U_SH)
        Q = basis_fold(np.einsum("uf,rfo->ruo", u_feature[sl], weight_v))
        q_stat = np.ascontiguousarray(
            Q.reshape(R, UC, 128, F).transpose(2, 1, 0, 3).reshape(128, -1)
        ).astype(np.float16)
        a = adj16[sl]
        in_maps.append({
            "adj_h": np.ascontiguousarray(a),
            "adjt_h": np.ascontiguousarray(a.T),
            "q_stat_h": q_stat,
            "p_stat_h": p_stat,
        })
    return in_maps


def kernel(adj, u_feature, v_feature, weight_u, weight_v, _trace=False):
    from concourse import bass_utils

    if "nc" not in _CACHE:
        _CACHE["nc"] = _build()
    nc = _CACHE["nc"]

    in_maps = _host_prep(adj, u_feature, v_feature, weight_u, weight_v)
    res = bass_utils.run_bass_kernel_spmd(
        nc, in_maps, core_ids=list(range(N_CORES)), trace=_trace)
    _CACHE["last_result"] = res

    adj = np.asarray(adj)
    deg_u = (adj > 0).sum(axis=1).astype(np.float64)
    deg_v = (adj > 0).sum(axis=0).astype(np.float64)
    d_u = np.where(deg_u > 0, 1.0 / np.maximum(deg_u, 0.5), 0.0)
    d_v = np.where(deg_v > 0, 1.0 / np.maximum(deg_v, 0.5), 0.0)

    # out_u partial per core: [128, 1024] fp16, cols h*512.. = u-half h
    # rows [f0:32 even bases | f32:64 even | f0:32 odd | f32:64 odd]
    outs = []
    for c in range(N_CORES):
        x = res.results[c]["out_u_part"].astype(np.float32)  # [128, 1024]
        ut = (x[0:32] + x[64:96], x[32:64] + x[96:128])
        outs.append(np.concatenate(ut, axis=0))              # [64, 1024]
    out_uT = np.concatenate(outs, axis=1)                    # [64, 8192]
    out_u = np.maximum(out_uT.T * d_u[:, None], 0.0).astype(np.float32)

    acc = np.zeros((128, N_V), np.float64)
    for c in range(N_CORES):
        acc += res.results[c]["out_v_part"].astype(np.float64)
    out_vT = np.concatenate(
        [acc[0:32] + acc[64:96], acc[32:64] + acc[96:128]], axis=0)
    out_v = np.maximum(out_vT.T * d_v[:, None], 0.0).astype(np.float32)
    return out_u, out_v


# revision 9
# speedup vs baseline: 1.0364x; 1.0045x over previous
"""Bipartite graph convolution (GCMC-style) Trainium2 kernel, 8-core SPMD.

Math (reference): per-rating masks M_r = (adj == r), r=1..5,
  out_u = relu(d_u * sum_r (M_r @ v_feat) @ W_u[r]),  d_u = 1/deg_u
  out_v = relu(d_v * sum_r (M_r.T @ u_feat) @ W_v[r]), d_v = 1/deg_v

Device formulation (per core, u-rows sharded 1024/core), v3:
  Fold weights on host: P_b = v_feat @ W~_u[b]  [8192, 64] per basis b,
  Q_b = u_shard @ W~_v[b] [1024, 64], basis {adj, M2, M3, ramp4, M5}
  (adj = sum_r r*M_r) so only 4 on-chip mask passes per orientation,
  all on DVE (is_equal / subtract+max chains run in 4x mode ~1.2T elem/s).

  PE uses feature-stationary matmuls with 4-way column tiling: per span,
  4 concurrent MMs with 32-col stationaries (two bases x two 32-feature
  halves) stream 512-wide mask columns -> 4 moving cols/cycle, full
  128x128 array utilization, ~216ns per span. Output is feature-major
  [f, cols] in PSUM; all bases accumulate into one bank (basis pairs
  split across partition rows 0:64 / 64:128, summed on host). A dummy
  start=True matmul with a zero moving tile owns each bank's has_written
  clear (col-tiled groups run concurrently, so no real MM may clear).

  Phase 1 (out_v partial): adj row-tiles streamed [128u, 2048v] per
  (vbg, uc-pair); per v-block of 512, one PSUM bank accumulates over all
  uc and bases; 4 v-block groups alternate between two 4-bank PSUM sets
  so ACT evacuation overlaps the next group's accumulation. Partial
  [128, 8192] fp16 -> host (all-reduce over cores + halves-add + deg +
  relu). Phase 2 (out_u): adjT tiles streamed [128v, 1024u]; 2 PSUM
  banks (u-halves) accumulate over all 64 v-tiles; partial [128, 1024]
  fp16 -> host. Phase-2 tiles for the first few v-tiles prefetch during
  phase 1's last group to hide the transition.
"""

import numpy as np
import sys

sys.path.insert(0, "/opt/trn_rl_repo")

N_U, N_V = 8192, 8192
F = 64
R = 5
N_CORES = 8
U_SH = N_U // N_CORES          # 1024 rows per core
UC = U_SH // 128               # 8 u-chunks per core
VC = N_V // 128                # 64 v-chunks
VBW = 512                      # v-block width (phase 1 psum bank)
VBG = 4                        # v-block groups
HB = 4                         # v-blocks per group (psum bank set size)
W1 = N_V // VBG                # 2048 adj cols per phase-1 slice
PF2 = 3                        # phase-2 v-tiles prefetched during phase 1

_CACHE = {}


def _build():
    import concourse.bass as bass
    import concourse.bacc as bacc
    import concourse.mybir as mybir
    import concourse.tile as tile

    dt = mybir.dt
    eq = mybir.AluOpType.is_equal
    sub = mybir.AluOpType.subtract
    mx = mybir.AluOpType.max
    RELU = mybir.ActivationFunctionType.Relu

    nc = bacc.Bacc("TRN2", target_bir_lowering=False, debug=False,
                   num_devices=N_CORES)

    adj_h = nc.dram_tensor("adj_h", [U_SH, N_V], dt.float16,
                           kind="ExternalInput").ap()
    adjt_h = nc.dram_tensor("adjt_h", [N_V, U_SH], dt.float16,
                            kind="ExternalInput").ap()
    # stationaries: per-chunk feature projections, 5 bases x 64 features
    q_stat_h = nc.dram_tensor("q_stat_h", [128, UC * R * F], dt.float16,
                              kind="ExternalInput").ap()
    p_stat_h = nc.dram_tensor("p_stat_h", [128, VC * R * F], dt.float16,
                              kind="ExternalInput").ap()
    out_u = nc.dram_tensor("out_u_part", [128, 2 * VBW], dt.float16,
                           kind="ExternalOutput").ap()
    out_v = nc.dram_tensor("out_v_part", [128, N_V], dt.float16,
                           kind="ExternalOutput").ap()

    with tile.TileContext(nc) as tc:
        with tc.tile_pool(name="consts", bufs=1) as cons, \
             tc.tile_pool(name="fin", bufs=4) as fin:

            zt = cons.tile([128, VBW], dt.float16, tag="zt")
            nc.gpsimd.memset(zt[:], 0.0)
            q_t = cons.tile([128, UC * R * F], dt.float16, tag="q")
            nc.sync.dma_start(q_t[:], q_stat_h[:])

            def gen_masks(pool, src, W, key):
                """4 basis tiles from src [128, W], all on DVE: M2/M3/M5
                is_equal, ramp4 = (a-3) max 0. Accessor b, lo, hi -> AP,
                basis order {0:adj, 1:M2, 2:M3, 3:r4, 4:M5}."""
                mt = pool.tile([128, 4 * W], dt.float16, tag=f"mt{key[0]}",
                               name=f"mt{key}")
                nc.vector.tensor_scalar(mt[:, 0:W], src, 2.0, None, op0=eq)
                nc.vector.tensor_scalar(mt[:, W:2 * W], src, 3.0, None,
                                        op0=eq)
                nc.vector.tensor_scalar(mt[:, 2 * W:3 * W], src, 3.0, 0.0,
                                        op0=sub, op1=mx)
                nc.vector.tensor_scalar(mt[:, 3 * W:4 * W], src, 5.0, None,
                                        op0=eq)

                def basis(b, lo, hi):
                    if b == 0:
                        return src[:, lo:hi]
                    return mt[:, (b - 1) * W + lo:(b - 1) * W + hi]
                return basis

            def clear_bank(ps_tile):
                nc.tensor.matmul(ps_tile[:], zt[:, 0:128], zt[:],
                                 start=True, stop=False,
                                 skip_group_check=True)

            def mm_grp(ps_tile, g, lhsT, rhs, stop=False):
                nc.tensor.matmul(ps_tile[32 * g:32 * (g + 1), :],
                                 lhsT, rhs,
                                 start=False, stop=stop,
                                 tile_position=(0, 32 * g),
                                 skip_group_check=True)

            def span(ps_tile, rhs_e, lhs_e, rhs_o, lhs_o, stop=False):
                """4-MM col-tiled span: groups 0/1 = even-basis feature
                halves vs rhs_e, groups 2/3 = odd basis vs rhs_o.
                lhs_*: (stationary tile, col offset of 64-wide block)."""
                (te, ce), (to, co) = lhs_e, lhs_o
                for g in range(4):
                    t, c0 = (te, ce) if g < 2 else (to, co)
                    rhs = rhs_e if g < 2 else rhs_o
                    fh = g % 2
                    mm_grp(ps_tile, g, t[:, c0 + 32 * fh:c0 + 32 * (fh + 1)],
                           rhs, stop=(stop and g == 3))

            # phase-2 SBUF pools opened early so the transition prefetches
            mpoolA = tc.tile_pool(name="maskA", bufs=PF2)
            mpa = mpoolA.__enter__()
            pstp = tc.tile_pool(name="pstream", bufs=PF2)
            pst = pstp.__enter__()

            p2_tiles = {}

            def p2_fetch(vc):
                if vc in p2_tiles:
                    return p2_tiles[vc]
                at = mpa.tile([128, U_SH], dt.float16, tag="adjt",
                              name=f"at{vc}")
                nc.sync.dma_start(at[:], adjt_h[vc * 128:(vc + 1) * 128, :])
                pt = pst.tile([128, R * F], dt.float16, tag="pstat",
                              name=f"pt{vc}")
                nc.sync.dma_start(pt[:],
                                  p_stat_h[:, vc * R * F:(vc + 1) * R * F])
                p2_tiles[vc] = (at, pt)
                return at, pt

            # ---------------- Phase 1: out_v partial ----------------
            pspB = tc.tile_pool(name="psumB", bufs=1, space="PSUM")
            psp = pspB.__enter__()
            mpoolB = tc.tile_pool(name="maskB", bufs=2)
            mpb = mpoolB.__enter__()
            ps_v = [psp.tile([128, VBW], dt.float32, tag=f"psv{k}",
                             name=f"psv{k}") for k in range(2 * HB)]

            for vbg in range(VBG):
                bset = ps_v[(vbg % 2) * HB:(vbg % 2) * HB + HB]
                for vb in range(HB):
                    clear_bank(bset[vb])
                for ucp in range(UC // 2):
                    masks = []
                    for i in range(2):
                        uc = 2 * ucp + i
                        at = mpb.tile([128, W1], dt.float16,
                                      tag=f"adj{i}", name=f"a{vbg}_{uc}")
                        nc.sync.dma_start(
                            at[:], adj_h[uc * 128:(uc + 1) * 128,
                                         vbg * W1:(vbg + 1) * W1])
                        basis = gen_masks(mpb, at[:], W1, (i, vbg, uc))
                        masks.append((uc, basis))
                    last_ucp = ucp == UC // 2 - 1
                    for vb in range(HB):
                        lo, hi = vb * VBW, (vb + 1) * VBW
                        for pr in range(2):  # pairs (adj,M2), (M3,r4)
                            for uc, basis in masks:
                                qc = uc * R * F
                                span(bset[vb],
                                     basis(2 * pr, lo, hi),
                                     (q_t, qc + (2 * pr) * F),
                                     basis(2 * pr + 1, lo, hi),
                                     (q_t, qc + (2 * pr + 1) * F))
                        # pair3: M5 of uc-even on grp 0/1, uc-odd on 2/3
                        (uca, ba), (ucb, bb) = masks
                        span(bset[vb],
                             ba(4, lo, hi), (q_t, uca * R * F + 4 * F),
                             bb(4, lo, hi), (q_t, ucb * R * F + 4 * F),
                             stop=last_ucp)
                if vbg == VBG - 1:
                    for vc in range(PF2):
                        p2_fetch(vc)
                # evacuate this vbg's banks (overlaps next vbg's spans)
                for vb in range(HB):
                    ev = fin.tile([128, VBW], dt.float16, tag="evacv",
                                  name=f"evv{vbg}_{vb}")
                    nc.scalar.copy(ev[:], bset[vb][:])
                    nc.sync.dma_start(
                        out_v[:, vbg * W1 + vb * VBW:
                              vbg * W1 + (vb + 1) * VBW], ev[:])
            mpoolB.__exit__(None, None, None)
            pspB.__exit__(None, None, None)

            # ---------------- Phase 2: out_u partial ----------------
            pspA = tc.tile_pool(name="psumA", bufs=1, space="PSUM")
            pspa = pspA.__enter__()
            ps_u = [pspa.tile([128, VBW], dt.float32, tag=f"psu{h}",
                              name=f"psu{h}") for h in range(2)]
            for h in range(2):
                clear_bank(ps_u[h])
            for vc in range(VC):
                at, pt = p2_fetch(vc)
                if vc + PF2 < VC:
                    p2_fetch(vc + PF2)
                basis = gen_masks(mpa, at[:], U_SH, ("p", vc))
                last = vc == VC - 1
                for h in range(2):
                    lo, hi = h * VBW, (h + 1) * VBW
                    for pr in range(2):
                        span(ps_u[h],
                             basis(2 * pr, lo, hi), (pt, (2 * pr) * F),
                             basis(2 * pr + 1, lo, hi),
                             (pt, (2 * pr + 1) * F))
                # pair3: M5 u-half0 on grp 0/1 (bank 0 rows 0:64),
                #        M5 u-half1 on grp 2/3 (bank 1 rows 64:128)
                for g in range(4):
                    h = g // 2
                    fh = g % 2
                    mm_grp(ps_u[h], g,
                           pt[:, 4 * F + 32 * fh:4 * F + 32 * (fh + 1)],
                           basis(4, h * VBW, (h + 1) * VBW),
                           stop=(last and fh == 1))
            for h in range(2):
                ev = fin.tile([128, VBW], dt.float16, tag="evacu",
                              name=f"evu{h}")
                nc.scalar.copy(ev[:], ps_u[h][:])
                nc.sync.dma_start(out_u[:, h * VBW:(h + 1) * VBW], ev[:])
            pstp.__exit__(None, None, None)
            mpoolA.__exit__(None, None, None)
            pspA.__exit__(None, None, None)

    nc.compile()
    return nc


def _host_prep(adj, u_feature, v_feature, weight_u, weight_v):
    adj = np.asarray(adj)
    u_feature = np.asarray(u_feature, dtype=np.float32)
    v_feature = np.asarray(v_feature, dtype=np.float32)
    weight_u = np.asarray(weight_u, dtype=np.float32)
    weight_v = np.asarray(weight_v, dtype=np.float32)

    adj16 = adj.astype(np.float16)

    def basis_fold(X):
        """X [R, n, F] per-rating projections -> basis projections for
        {adj, M2, M3, ramp4, M5}: sum_r M_r X_r = a*X1 + M2*(X2-2X1)
        + M3*(X3-3X1) + r4*(X4-4X1) + M5*(X5+3X1-2X4)   (r4(5)=2)."""
        B = np.empty_like(X)
        B[0] = X[0]
        B[1] = X[1] - 2 * X[0]
        B[2] = X[2] - 3 * X[0]
        B[3] = X[3] - 4 * X[0]
        B[4] = X[4] + 3 * X[0] - 2 * X[3]
        return B

    # P_b = v_feat @ W_u basis-folded  [R, N_V, F]
    P = basis_fold(np.einsum("vf,rfo->rvo", v_feature, weight_u))
    # p_stat[p, (vc*R+b)*F + j] = P[b, vc*128+p, j]
    p_stat = np.ascontiguousarray(
        P.reshape(R, VC, 128, F).transpose(2, 1, 0, 3).reshape(128, -1)
    ).astype(np.float16)

    in_maps = []
    for c in range(N_CORES):
        sl = slice(c * U_SH, (c + 1) * U_SH)
        Q = basis_fold(np.einsum("uf,rfo->ruo", u_feature[sl], weight_v))
        q_stat = np.ascontiguousarray(
            Q.reshape(R, UC, 128, F).transpose(2, 1, 0, 3).reshape(128, -1)
        ).astype(np.float16)
        a = adj16[sl]
        in_maps.append({
            "adj_h": np.ascontiguousarray(a),
            "adjt_h": np.ascontiguousarray(a.T),
            "q_stat_h": q_stat,
            "p_stat_h": p_stat,
        })
    return in_maps


def kernel(adj, u_feature, v_feature, weight_u, weight_v, _trace=False):
    from concourse import bass_utils

    if "nc" not in _CACHE:
        _CACHE["nc"] = _build()
    nc = _CACHE["nc"]

    in_maps = _host_prep(adj, u_feature, v_feature, weight_u, weight_v)
    res = bass_utils.run_bass_kernel_spmd(
        nc, in_maps, core_ids=list(range(N_CORES)), trace=_trace)
    _CACHE["last_result"] = res

    adj = np.asarray(adj)
    deg_u = (adj > 0).sum(axis=1).astype(np.float64)
    deg_v = (adj > 0).sum(axis=0).astype(np.float64)
    d_u = np.where(deg_u > 0, 1.0 / np.maximum(deg_u, 0.5), 0.0)
    d_v = np.where(deg_v > 0, 1.0 / np.maximum(deg_v, 0.5), 0.0)

    # out_u partial per core: [128, 1024] fp16, cols h*512.. = u-half h
    # rows [f0:32 even bases | f32:64 even | f0:32 odd | f32:64 odd]
    outs = []
    for c in range(N_CORES):
        x = res.results[c]["out_u_part"].astype(np.float32)  # [128, 1024]
        ut = (x[0:32] + x[64:96], x[32:64] + x[96:128])
        outs.append(np.concatenate(ut, axis=0))              # [64, 1024]
    out_uT = np.concatenate(outs, axis=1)                    # [64, 8192]
    out_u = np.maximum(out_uT.T * d_u[:, None], 0.0).astype(np.float32)

    acc = np.zeros((128, N_V), np.float64)
    for c in range(N_CORES):
        acc += res.results[c]["out_v_part"].astype(np.float64)
    out_vT = np.concatenate(
        [acc[0:32] + acc[64:96], acc[32:64] + acc[96:128]], axis=0)
    out_v = np.maximum(out_vT.T * d_v[:, None], 0.0).astype(np.float32)
    return out_u, out_v
